# revision 7
# baseline (speedup 1.0000x reference)
"""DyadicQALoRA fused kernel for Trainium2 (8 NeuronCores).

Computes, for x:[B,S,Din], weight:[Dout,Din], bias:[Dout], lora_A:[Din,16],
lora_B:[16,Dout]:

    x_q, x_scale = per-token int8 absmax quant(x)        (exact RNE rounding)
    w_q, w_scale = ternary absmean quant(weight)
    a_q, a_s     = per-tensor int8 quant(lora_A)
    b_q, b_s     = per-tensor int8 quant(lora_B)
    out = (x_q @ w_q.T) * (w_scale*x_scale) + bias
        + ((x_q @ a_q) @ b_q) * (x_scale*a_s*b_s*2.0)

Sharding: 2-D tensor/data hybrid over 8 cores - 4 token groups x 2
out-feature groups.  The only collective is a 1-scalar AllReduce for the
global absmean weight scale (each core reduces a disjoint 1/8 row shard).

Performance structure (v2):
  - Weight prep is fully software-pipelined across engines
    (DMA -> vector TS x2 -> scalar round-to-bf16 -> xbar transpose ->
    gpsimd cast-to-fp8 copy), interleaved into the emission of the first
    token blocks so matmuls start as soon as the first 512-out-feature
    chunk of w_qT is ready (instead of after the entire prep).
  - Main loop is chunk-major: 32 consecutive matmuls accumulate into the
    SAME psum bank (hw cadence 216ns/matmul vs 259ns when round-robining
    psum banks), with a per-chunk epilogue so psum frees early.
  - LoRA stage 2 is a single K=32 matmul per chunk: lhsT = [hi;lo] bf16
    split of xa*c2 (c2 = a_s*b_s*2/w_scale), rhs = [b_q;b_q], accumulated
    onto the base psum so the epilogue is out = psum*(x_scale*w_scale)+bias.
"""

import os
import sys
import functools

import numpy as np

for _p in ("/opt/trn_rl_repo", "/root/.axon_site/_ro/trn_rl_repo"):
    if os.path.isdir(_p) and _p not in sys.path:
        sys.path.insert(0, _p)

import ml_dtypes  # noqa: E402
import concourse.bass as bass  # noqa: E402
import concourse.mybir as mybir  # noqa: E402
from concourse import bacc  # noqa: E402
from concourse import bass_isa  # noqa: E402
from concourse import tile  # noqa: E402

F32 = mybir.dt.float32
BF16 = mybir.dt.bfloat16
FP8 = mybir.dt.float8e4

MAGIC = 12582912.0  # 1.5 * 2**23 : fp32 add/sub gives exact RNE round
QMAX = 127.0
EPS = 1e-6
SCALING = 2.0  # alpha/rank = 32/16
N_CORES = 8
R_TOK = 4  # token groups
C_OUT = 2  # out-feature groups


def build_nc(TOK, DIN, DOUT_C, WSC_ROWS, N_FULL_W, RANK=16):
    """Build the per-core (SPMD) Bass program."""
    assert TOK % 128 == 0 and DIN % 256 == 0 and DOUT_C % 512 == 0
    KT = DIN // 128          # k tiles (32)
    MT = TOK // 128          # token tiles (16)
    NCH = DOUT_C // 512      # 512-wide out-feature chunks (4)
    WT = DOUT_C // 128       # weight row tiles (16)
    WSCT = WSC_ROWS // 128   # weight-scale shard tiles (4)
    DH = DIN // 2            # x half width (2048)
    KTH = KT // 2
    DOUT_FULL = N_FULL_W // DIN

    nc = bacc.Bacc(
        "TRN2", target_bir_lowering=False, debug=False, num_devices=N_CORES,
    )

    x_in = nc.dram_tensor("x_in", [TOK, DIN], F32, kind="ExternalInput")
    w_in = nc.dram_tensor("w_in", [DOUT_C, DIN], F32, kind="ExternalInput")
    wsc_in = nc.dram_tensor("wsc_in", [WSC_ROWS, DIN], F32, kind="ExternalInput")
    bias_in = nc.dram_tensor("bias_in", [1, DOUT_C], F32, kind="ExternalInput")
    la_in = nc.dram_tensor("la_in", [DIN, RANK], F32, kind="ExternalInput")
    lbf_in = nc.dram_tensor("lbf_in", [16, DOUT_FULL], F32, kind="ExternalInput")
    lb_in = nc.dram_tensor("lb_in", [16, DOUT_C], F32, kind="ExternalInput")
    ident_in = nc.dram_tensor("ident_in", [128, 128], BF16, kind="ExternalInput")
    out_d = nc.dram_tensor("out", [TOK, DOUT_C], F32, kind="ExternalOutput")

    with tile.TileContext(nc) as tc:
        with (
            tc.tile_pool(name="const", bufs=1) as cpool,
            tc.tile_pool(name="wq", bufs=1) as wqpool,
            tc.tile_pool(name="wf32", bufs=2) as wfpool,
            tc.tile_pool(name="wqb", bufs=2) as wbpool,
            tc.tile_pool(name="wqTs", bufs=2) as wspool,
            tc.tile_pool(name="xf32", bufs=2) as xfpool,
            tc.tile_pool(name="xq", bufs=2) as xqpool,
            tc.tile_pool(name="xqT", bufs=3) as xtpool,
            tc.tile_pool(name="uout", bufs=4) as upool,
            tc.tile_pool(name="small", bufs=4) as spool,
            tc.tile_pool(name="psmain", bufs=5, space="PSUM") as ppool,
            tc.tile_pool(name="psxa", bufs=2, space="PSUM") as papool,
            tc.tile_pool(name="psptr", bufs=1, space="PSUM") as ptpool,
            tc.tile_pool(name="dram", bufs=1, space="DRAM") as dpool,
        ):
            # ================= constants =================
            ident = cpool.tile([128, 128], BF16, tag="ident")
            nc.sync.dma_start(ident[:], ident_in[:])

            bias_row = wfpool.tile([1, DOUT_C], F32, tag="wf", name="bias_row")
            nc.sync.dma_start(bias_row[:], bias_in[:])
            bias_b = cpool.tile([128, DOUT_C], F32, tag="bias_b")
            nc.gpsimd.partition_broadcast(bias_b[:], bias_row[:])

            # ============ global |w| mean partials (wsc shard) ============
            wsums = cpool.tile([128, WSCT], F32, tag="wsums")
            wsc_tiles = []
            for t in range(WSCT):
                wst = wfpool.tile([128, DIN], F32, tag="wf", name=f"wsc{t}")
                eng = nc.sync if t % 2 == 0 else nc.scalar
                eng.dma_start(wst[:], wsc_in[t * 128:(t + 1) * 128, :])
                wsc_tiles.append(wst)
            for t in range(WSCT):
                nc.vector.tensor_reduce(
                    wsums[:, t:t + 1], wsc_tiles[t][:], axis=mybir.AxisListType.X,
                    op=mybir.AluOpType.add, apply_absolute_value=True,
                )
            wsum_p = cpool.tile([128, 1], F32, tag="wsum_p")
            nc.vector.tensor_reduce(
                wsum_p[:], wsums[:], axis=mybir.AxisListType.X,
                op=mybir.AluOpType.add,
            )
            wsum_b = cpool.tile([128, 1], F32, tag="wsum_b")
            nc.gpsimd.partition_all_reduce(
                wsum_b[:], wsum_p[:], channels=128,
                reduce_op=bass_isa.ReduceOp.add,
            )
            cc_in = dpool.tile([1, 1], F32)
            cc_out = dpool.tile([1, 1], F32)
            nc.sync.dma_start(cc_in[:], wsum_b[0:1, :])

            # ============ lora prep (no dependence on w_scale) ============
            la_s = wfpool.tile([128, KT, RANK], F32, tag="wf", name="la_s")
            nc.sync.dma_start(
                la_s[:], la_in.rearrange("(kt p) r -> p kt r", p=128)
            )
            amax0 = cpool.tile([128, 1], F32, tag="amax0")
            nc.vector.tensor_reduce(
                amax0[:], la_s[:], axis=mybir.AxisListType.XY,
                op=mybir.AluOpType.max, apply_absolute_value=True,
            )
            amax = cpool.tile([128, 1], F32, tag="amax")
            nc.gpsimd.partition_all_reduce(
                amax[:], amax0[:], channels=128, reduce_op=bass_isa.ReduceOp.max,
            )
            amax_c = cpool.tile([128, 1], F32, tag="amax_c")
            nc.vector.tensor_scalar(
                amax_c[:], amax[:], EPS, None, op0=mybir.AluOpType.max,
            )
            ia = cpool.tile([128, 1], F32, tag="ia")
            nc.vector.reciprocal(ia[:], amax_c[:])
            ia127 = cpool.tile([128, 1], F32, tag="ia127")
            nc.vector.tensor_scalar(
                ia127[:], ia[:], QMAX, None, op0=mybir.AluOpType.mult,
            )
            a_sc = cpool.tile([128, 1], F32, tag="a_sc")  # a_s = amax/127
            nc.vector.tensor_scalar(
                a_sc[:], amax_c[:], 1.0 / QMAX, None, op0=mybir.AluOpType.mult,
            )
            nc.vector.tensor_scalar(
                la_s[:], la_s[:], ia127[:], MAGIC,
                op0=mybir.AluOpType.mult, op1=mybir.AluOpType.add,
            )
            a_q = cpool.tile([128, KT, RANK], BF16, tag="a_q")
            nc.vector.tensor_scalar(
                a_q[:], la_s[:], -MAGIC, None, op0=mybir.AluOpType.add,
            )

            # lora_B absmax over the FULL tensor
            lbf_s = wfpool.tile([16, DOUT_FULL], F32, tag="wf", name="lbf_s")
            nc.sync.dma_start(lbf_s[:], lbf_in[:])
            bmax0 = cpool.tile([16, 1], F32, tag="bmax0")
            nc.vector.tensor_reduce(
                bmax0[:], lbf_s[:], axis=mybir.AxisListType.X,
                op=mybir.AluOpType.max, apply_absolute_value=True,
            )
            bmax = cpool.tile([16, 1], F32, tag="bmax")
            nc.gpsimd.partition_all_reduce(
                bmax[:], bmax0[:], channels=16, reduce_op=bass_isa.ReduceOp.max,
            )
            bmax_c = cpool.tile([16, 1], F32, tag="bmax_c")
            nc.vector.tensor_scalar(
                bmax_c[:], bmax[:], EPS, None, op0=mybir.AluOpType.max,
            )
            bmax_b = cpool.tile([128, 1], F32, tag="bmax_b")
            nc.gpsimd.partition_broadcast(bmax_b[:], bmax_c[0:1, :])
            ib = cpool.tile([128, 1], F32, tag="ib")
            nc.vector.reciprocal(ib[:], bmax_b[:])
            ib127 = cpool.tile([128, 1], F32, tag="ib127")
            nc.vector.tensor_scalar(
                ib127[:], ib[:], QMAX, None, op0=mybir.AluOpType.mult,
            )
            b_sc = cpool.tile([128, 1], F32, tag="b_sc")
            nc.vector.tensor_scalar(
                b_sc[:], bmax_b[:], 1.0 / QMAX, None, op0=mybir.AluOpType.mult,
            )
            # b_q2 = [b_q ; b_q]  (stacked for the K=32 lora stage-2 matmul)
            lb2_s = wfpool.tile([32, DOUT_C], F32, tag="wf", name="lb2_s")
            nc.sync.dma_start(lb2_s[0:16, :], lb_in[:])
            nc.sync.dma_start(lb2_s[16:32, :], lb_in[:])
            nc.vector.tensor_scalar(
                lb2_s[:], lb2_s[:], ib127[0:32, :], MAGIC,
                op0=mybir.AluOpType.mult, op1=mybir.AluOpType.add,
            )
            b_q2 = cpool.tile([32, DOUT_C], BF16, tag="b_q2")
            nc.vector.tensor_scalar(
                b_q2[:], lb2_s[:], -MAGIC, None, op0=mybir.AluOpType.add,
            )

            # ============ x pipeline ============
            xqT_l = [None] * MT
            xs_t_l = [None] * MT
            xsws_l = [None] * MT

            def emit_xsws(m):
                xsws = spool.tile([128, 1], F32, tag="xsws", name=f"xsws{m}")
                nc.vector.tensor_tensor(
                    xsws[:], xs_t_l[m][:], ws_t[:], op=mybir.AluOpType.mult,
                )
                xsws_l[m] = xsws

            def emit_x(m, defer_xsws=False):
                sxh = spool.tile([128, 2], F32, tag="sxh", name=f"sxh{m}")
                xqT = xtpool.tile([128, KT, 128], BF16, tag="xqT",
                                  name=f"xqT{m}")
                sx = spool.tile([128, 1], F32, tag="sx", name=f"sx{m}")
                sxc = spool.tile([128, 1], F32, tag="sxc", name=f"sxc{m}")
                xs_t = spool.tile([128, 1], F32, tag="xs_t", name=f"xs_t{m}")
                ix = spool.tile([128, 1], F32, tag="ix", name=f"ix{m}")
                xhs = []
                for h in range(2):
                    xh = xfpool.tile([128, DH], F32, tag="xf",
                                     name=f"x{m}h{h}")
                    nc.sync.dma_start(
                        xh[:], x_in[m * 128:(m + 1) * 128, h * DH:(h + 1) * DH])
                    nc.vector.tensor_reduce(
                        sxh[:, h:h + 1], xh[:], axis=mybir.AxisListType.X,
                        op=mybir.AluOpType.max, apply_absolute_value=True,
                    )
                    xhs.append(xh)
                nc.vector.tensor_reduce(
                    sx[:], sxh[:], axis=mybir.AxisListType.X,
                    op=mybir.AluOpType.max,
                )
                nc.vector.tensor_scalar(
                    sxc[:], sx[:], EPS, None, op0=mybir.AluOpType.max,
                )
                nc.vector.tensor_scalar(
                    xs_t[:], sxc[:], 1.0 / QMAX, None, op0=mybir.AluOpType.mult,
                )
                nc.vector.reciprocal(ix[:], xs_t[:])
                xs_t_l[m] = xs_t
                if not defer_xsws:
                    emit_xsws(m)
                for h in range(2):
                    xh = xhs[h]
                    nc.vector.tensor_scalar(
                        xh[:], xh[:], ix[:], MAGIC,
                        op0=mybir.AluOpType.mult, op1=mybir.AluOpType.add,
                    )
                    xqh = xqpool.tile([128, DH], BF16, tag="xq",
                                      name=f"xq{m}h{h}")
                    nc.vector.tensor_scalar(
                        xqh[:], xh[:], -MAGIC, None, op0=mybir.AluOpType.add,
                    )
                    nc.scalar.dma_start(
                        xqT[:, h * KTH:(h + 1) * KTH, :], xqh[:],
                        transpose=True,
                    )
                xqT_l[m] = xqT

            # ============ lora stage-1 + hi/lo pieces ============
            pcT_l = [None] * MT

            def emit_lora1(m):
                pxa = papool.tile([128, RANK], F32, tag="pxa", name=f"pxa{m}")
                for k in range(KT):
                    nc.tensor.matmul(
                        pxa[:], xqT_l[m][:, k, :], a_q[:, k, :],
                        start=(k == 0), stop=(k == KT - 1),
                    )
                v_xa = spool.tile([128, RANK], F32, tag="v_xa", name=f"vxa{m}")
                nc.vector.tensor_scalar(
                    v_xa[:], pxa[:], c2[:], None, op0=mybir.AluOpType.mult,
                )
                pieces = spool.tile([128, 2 * RANK], BF16, tag="pieces",
                                    name=f"pieces{m}")
                nc.vector.tensor_copy(pieces[:, 0:RANK], v_xa[:])
                hi_f = spool.tile([128, RANK], F32, tag="hi_f", name=f"hif{m}")
                nc.vector.tensor_copy(hi_f[:], pieces[:, 0:RANK])
                nc.vector.tensor_tensor(
                    pieces[:, RANK:2 * RANK], v_xa[:], hi_f[:],
                    op=mybir.AluOpType.subtract,
                )
                ptp = ptpool.tile([2 * RANK, 128], BF16, tag="ptp",
                                  name=f"ptp{m}")
                nc.tensor.transpose(ptp[:], pieces[:], ident[:])
                pcT = spool.tile([2 * RANK, 128], BF16, tag="pcT",
                                 name=f"pcT{m}")
                nc.scalar.copy(pcT[:], ptp[:])
                pcT_l[m] = pcT

            # first two token tiles start loading/quantizing immediately
            # (ws_t does not exist yet -> defer their xsws products)
            emit_x(0, defer_xsws=True)
            emit_x(1, defer_xsws=True)

            # ============ AllReduce result pickup ============
            nc.gpsimd.collective_compute(
                "AllReduce", mybir.AluOpType.add,
                replica_groups=[list(range(N_CORES))],
                ins=[cc_in.opt()], outs=[cc_out.opt()],
            )
            wsg = cpool.tile([1, 1], F32, tag="wsg")
            # scalar queue is idle in the AR window; vector must stay free
            nc.scalar.dma_start(wsg[:], cc_out[:])
            wsg_b = cpool.tile([128, 1], F32, tag="wsg_b")
            nc.gpsimd.partition_broadcast(wsg_b[:], wsg[:])
            ws_t = cpool.tile([128, 1], F32, tag="ws_t")
            nc.vector.tensor_scalar(
                ws_t[:], wsg_b[:], 1.0 / float(N_FULL_W), EPS,
                op0=mybir.AluOpType.mult, op1=mybir.AluOpType.max,
            )
            inv_ws = cpool.tile([128, 1], F32, tag="inv_ws")
            nc.vector.reciprocal(inv_ws[:], ws_t[:])

            # c2 = a_s * b_s * SCALING / w_scale
            c2a = cpool.tile([128, 1], F32, tag="c2a")
            nc.vector.tensor_tensor(
                c2a[:], a_sc[:], b_sc[:], op=mybir.AluOpType.mult,
            )
            c2b = cpool.tile([128, 1], F32, tag="c2b")
            nc.vector.tensor_scalar(
                c2b[:], c2a[:], SCALING, None, op0=mybir.AluOpType.mult,
            )
            c2 = cpool.tile([128, 1], F32, tag="c2")
            nc.vector.tensor_tensor(
                c2[:], c2b[:], inv_ws[:], op=mybir.AluOpType.mult,
            )
            emit_xsws(0)
            emit_xsws(1)

            # ============ weight quant pipeline ============
            # wqT[c][p, sl, k, col] = w_q[dout=512c+128sl+col, din=128k+p]
            wqT = [
                wqpool.tile([128, 4, KT, 128], FP8, tag=f"wqT{c}",
                            name=f"wqT{c}")
                for c in range(NCH)
            ]

            def emit_w(j, trig_only=False):
                wt = wfpool.tile([128, DIN], F32, tag="wf", name=f"w{j}")
                nc.gpsimd.dma_start(wt[:], w_in[j * 128:(j + 1) * 128, :])
                return wt

            def emit_wquant(wt, j):
                nc.vector.tensor_scalar(
                    wt[:], wt[:], inv_ws[:], 1.49,
                    op0=mybir.AluOpType.mult, op1=mybir.AluOpType.min,
                )
                nc.vector.tensor_scalar(
                    wt[:], wt[:], -1.49, MAGIC,
                    op0=mybir.AluOpType.max, op1=mybir.AluOpType.add,
                )
                wqb = wbpool.tile([128, DIN], BF16, tag="wqb", name=f"wqb{j}")
                nc.scalar.activation(
                    wqb[:], wt[:], mybir.ActivationFunctionType.Copy,
                    bias=-MAGIC,
                )
                wqTs = wspool.tile([128, KT, 128], BF16, tag="wqTs",
                                   name=f"wqTs{j}")
                nc.sync.dma_start(wqTs[:], wqb[:], transpose=True)
                c, sl = j // 4, j % 4
                nc.gpsimd.tensor_copy(wqT[c][:, sl, :, :], wqTs[:])

            def emit_w_chain(j):
                emit_wquant(emit_w(j), j)

            # chunks 0 and 1 before the main loop; 2 and 3 interleaved into
            # token block 0 so their prep overlaps block 0's matmuls.
            for j in range(8):
                emit_w_chain(j)

            emit_x(2)
            emit_lora1(0)

            # ============ main loop (chunk-major) ============
            for m in range(MT):
                if m + 3 < MT:
                    emit_x(m + 3)
                for c in range(NCH):
                    ps = ppool.tile([128, 512], F32, tag="ps",
                                    name=f"ps{m}_{c}")
                    for k in range(KT):
                        nc.tensor.matmul(
                            ps[:], xqT_l[m][:, k, :], wqT[c][:, :, k, :],
                            start=(k == 0), stop=False,
                        )
                    nc.tensor.matmul(
                        ps[:], pcT_l[m][:], b_q2[:, c * 512:(c + 1) * 512],
                        start=False, stop=True,
                    )
                    u = upool.tile([128, 512], F32, tag="u", name=f"u{m}_{c}")
                    nc.scalar.activation(
                        u[:], ps[:], mybir.ActivationFunctionType.Copy,
                        bias=0.0, scale=xsws_l[m][:],
                    )
                    nc.vector.tensor_tensor(
                        u[:], u[:], bias_b[:, c * 512:(c + 1) * 512],
                        op=mybir.AluOpType.add,
                    )
                    nc.scalar.dma_start(
                        out_d[m * 128:(m + 1) * 128, c * 512:(c + 1) * 512],
                        u[:],
                    )
                    if m == 0 and c in (0, 2):
                        # finish weight prep under block-0 matmuls
                        for j in range(8 + (c // 2) * 4, 12 + (c // 2) * 4):
                            emit_w_chain(j)
                    if c == 1 and m + 1 < MT:
                        emit_lora1(m + 1)

    nc.compile()
    return nc


# ----------------------------------------------------------------------
# host-side wrapper
# ----------------------------------------------------------------------

@functools.lru_cache(maxsize=2)
def _get_nc(TOK, DIN, DOUT_C, WSC_ROWS, N_FULL_W):
    return build_nc(TOK, DIN, DOUT_C, WSC_ROWS, N_FULL_W)


def _prep(x, weight, bias, lora_A, lora_B):
    B, S, DIN = x.shape
    DOUT = weight.shape[0]
    NTOK = B * S
    assert NTOK % R_TOK == 0 and DOUT % C_OUT == 0 and DOUT % N_CORES == 0
    TOK = NTOK // R_TOK
    DOUT_C = DOUT // C_OUT
    WSC_ROWS = DOUT // N_CORES
    N_FULL_W = DOUT * DIN

    nc = _get_nc(TOK, DIN, DOUT_C, WSC_ROWS, N_FULL_W)

    x2 = np.ascontiguousarray(x.reshape(NTOK, DIN).astype(np.float32, copy=False))
    weight = np.ascontiguousarray(weight.astype(np.float32, copy=False))
    ident = np.eye(128, dtype=ml_dtypes.bfloat16)

    in_maps = []
    for core in range(N_CORES):
        i, j = core // C_OUT, core % C_OUT
        in_maps.append({
            "x_in": np.ascontiguousarray(x2[i * TOK:(i + 1) * TOK]),
            "w_in": np.ascontiguousarray(weight[j * DOUT_C:(j + 1) * DOUT_C]),
            "wsc_in": np.ascontiguousarray(
                weight[core * WSC_ROWS:(core + 1) * WSC_ROWS]),
            "bias_in": np.ascontiguousarray(
                bias[j * DOUT_C:(j + 1) * DOUT_C].reshape(1, DOUT_C)),
            "la_in": np.ascontiguousarray(lora_A.astype(np.float32, copy=False)),
            "lbf_in": np.ascontiguousarray(lora_B.astype(np.float32, copy=False)),
            "lb_in": np.ascontiguousarray(lora_B[:, j * DOUT_C:(j + 1) * DOUT_C]),
            "ident_in": ident,
        })
    return nc, in_maps, (B, S, NTOK, TOK, DOUT, DOUT_C)


def kernel(x, weight, bias, lora_A, lora_B):
    from concourse.bass_utils import run_bass_kernel_spmd

    nc, in_maps, (B, S, NTOK, TOK, DOUT, DOUT_C) = _prep(
        x, weight, bias, lora_A, lora_B)
    res = run_bass_kernel_spmd(nc, in_maps, core_ids=list(range(N_CORES)))

    out = np.empty((NTOK, DOUT), np.float32)
    for core in range(N_CORES):
        i, j = core // C_OUT, core % C_OUT
        out[i * TOK:(i + 1) * TOK, j * DOUT_C:(j + 1) * DOUT_C] = \
            res.results[core]["out"]
    return out.reshape(B, S, DOUT)


def _install_profile_shim():
    """Register the axon NTFF profile hook (antenv.axon_hooks is absent in
    this image; libaxon_pjrt.so supports the profile C ABI directly) and
    stub out the network-dependent artifact upload."""
    import types
    import ctypes
    import contextlib

    try:
        import antenv.axon_hooks  # noqa: F401
        have = True
    except ImportError:
        have = False
    if not have:
        so = "/opt/axon/libaxon_pjrt.so"
        lib = ctypes.CDLL(so)
        lib.axon_start_nrt_profile.argtypes = [
            ctypes.POINTER(ctypes.c_int64), ctypes.c_size_t]
        lib.axon_start_nrt_profile.restype = ctypes.c_int64
        lib.axon_stop_nrt_profile.argtypes = [ctypes.c_char_p]
        lib.axon_stop_nrt_profile.restype = ctypes.c_int64

        @contextlib.contextmanager
        def _hook(output_dir, device_ids):
            import jax
            jax.devices()
            if device_ids:
                ids = (ctypes.c_int64 * len(device_ids))(*device_ids)
                rc = lib.axon_start_nrt_profile(ids, len(device_ids))
            else:
                rc = lib.axon_start_nrt_profile(None, 0)
            if rc != 0:
                raise RuntimeError(f"axon_start_nrt_profile rc={rc}")
            try:
                yield
            finally:
                lib.axon_stop_nrt_profile(str(output_dir).encode())

        import antenv
        mod = types.ModuleType("antenv.axon_hooks")
        mod.get_axon_ntff_profile_hook = lambda: _hook
        mod.set_axon_ntff_profile_hook = lambda h: None
        sys.modules["antenv.axon_hooks"] = mod
        antenv.axon_hooks = mod

    from concourse import bass_utils
    bass_utils.upload_artifacts = lambda tmpdir: f"local:{tmpdir}"


def timed_run(inputs, trace_cores=None):
    """Run with NTFF tracing; returns max exec_time_ns across traced cores."""
    import tempfile
    _install_profile_shim()
    from concourse.bass_utils import run_bass_kernel_spmd

    nc, in_maps, _ = _prep(**inputs)
    res = run_bass_kernel_spmd(
        nc, in_maps, core_ids=list(range(N_CORES)), trace=True,
        trace_cores=trace_cores if trace_cores is not None
        else list(range(N_CORES)),
        tmpdir=tempfile.mkdtemp(prefix="dyadic_trace_"),
    )
    return res.exec_time_ns


# revision 15
# speedup vs baseline: 1.0642x; 1.0642x over previous
"""DyadicQALoRA fused kernel for Trainium2 (8 NeuronCores).

Computes, for x:[B,S,Din], weight:[Dout,Din], bias:[Dout], lora_A:[Din,16],
lora_B:[16,Dout]:

    x_q, x_scale = per-token int8 absmax quant(x)        (exact RNE rounding)
    w_q, w_scale = ternary absmean quant(weight)
    a_q, a_s     = per-tensor int8 quant(lora_A)
    b_q, b_s     = per-tensor int8 quant(lora_B)
    out = (x_q @ w_q.T) * (w_scale*x_scale) + bias
        + ((x_q @ a_q) @ b_q) * (x_scale*a_s*b_s*2.0)

Sharding: 2-D tensor/data hybrid over 8 cores - 4 token groups x 2
out-feature groups.  The only collective is a 1-scalar AllReduce for the
global absmean weight scale (each core reduces a disjoint 1/8 row shard).

Performance structure (v2):
  - Weight prep is fully software-pipelined across engines
    (DMA -> vector TS x2 -> scalar round-to-bf16 -> xbar transpose ->
    gpsimd cast-to-fp8 copy), interleaved into the emission of the first
    token blocks so matmuls start as soon as the first 512-out-feature
    chunk of w_qT is ready (instead of after the entire prep).
  - Main loop is chunk-major: 32 consecutive matmuls accumulate into the
    SAME psum bank (hw cadence 216ns/matmul vs 259ns when round-robining
    psum banks), with a per-chunk epilogue so psum frees early.
  - LoRA stage 2 is a single K=32 matmul per chunk: lhsT = [hi;lo] bf16
    split of xa*c2 (c2 = a_s*b_s*2/w_scale), rhs = [b_q;b_q], accumulated
    onto the base psum so the epilogue is out = psum*(x_scale*w_scale)+bias.
"""

import os
import sys
import functools

import numpy as np

for _p in ("/opt/trn_rl_repo", "/root/.axon_site/_ro/trn_rl_repo"):
    if os.path.isdir(_p) and _p not in sys.path:
        sys.path.insert(0, _p)

import ml_dtypes  # noqa: E402
import concourse.bass as bass  # noqa: E402
import concourse.mybir as mybir  # noqa: E402
from concourse import bacc  # noqa: E402
from concourse import bass_isa  # noqa: E402
from concourse import tile  # noqa: E402

F32 = mybir.dt.float32
BF16 = mybir.dt.bfloat16
FP8 = mybir.dt.float8e4

MAGIC = 12582912.0  # 1.5 * 2**23 : fp32 add/sub gives exact RNE round
QMAX = 127.0
EPS = 1e-6
SCALING = 2.0  # alpha/rank = 32/16
N_CORES = 8
R_TOK = 4  # token groups
C_OUT = 2  # out-feature groups


def build_nc(TOK, DIN, DOUT_C, WSC_ROWS, N_FULL_W, RANK=16):
    """Build the per-core (SPMD) Bass program."""
    assert TOK % 128 == 0 and DIN % 256 == 0 and DOUT_C % 512 == 0
    KT = DIN // 128          # k tiles (32)
    MT = TOK // 128          # token tiles (16)
    NCH = DOUT_C // 512      # 512-wide out-feature chunks (4)
    WT = DOUT_C // 128       # weight row tiles (16)
    WSCT = WSC_ROWS // 128   # weight-scale shard tiles (4)
    DH = DIN // 2            # x half width (2048)
    KTH = KT // 2
    DOUT_FULL = N_FULL_W // DIN
    NWAVE = 4                # token tiles consumed per chunk wave
    OFF = 192.0              # bf16-space int offset (ulp=1 in [128,256))

    nc = bacc.Bacc(
        "TRN2", target_bir_lowering=False, debug=False, num_devices=N_CORES,
    )

    x_in = nc.dram_tensor("x_in", [TOK, DIN], F32, kind="ExternalInput")
    w_in = nc.dram_tensor("w_in", [DOUT_C, DIN], F32, kind="ExternalInput")
    wsc_in = nc.dram_tensor("wsc_in", [WSC_ROWS, DIN], F32, kind="ExternalInput")
    bias_in = nc.dram_tensor("bias_in", [1, DOUT_C], F32, kind="ExternalInput")
    la_in = nc.dram_tensor("la_in", [DIN, RANK], F32, kind="ExternalInput")
    lbf_in = nc.dram_tensor("lbf_in", [16, DOUT_FULL], F32, kind="ExternalInput")
    lb_in = nc.dram_tensor("lb_in", [16, DOUT_C], F32, kind="ExternalInput")
    ident_in = nc.dram_tensor("ident_in", [128, 128], BF16, kind="ExternalInput")
    out_d = nc.dram_tensor("out", [TOK, DOUT_C], F32, kind="ExternalOutput")

    with tile.TileContext(nc) as tc:
        with (
            tc.tile_pool(name="const", bufs=1) as cpool,
            tc.tile_pool(name="wq", bufs=1) as wqpool,
            tc.tile_pool(name="wf32", bufs=2) as wfpool,
            tc.tile_pool(name="wqb", bufs=1) as wbpool,
            tc.tile_pool(name="wqTs", bufs=1) as wspool,
            tc.tile_pool(name="xf32", bufs=2) as xfpool,
            tc.tile_pool(name="xq", bufs=2) as xqpool,
            tc.tile_pool(name="xqT", bufs=6) as xtpool,
            tc.tile_pool(name="uout", bufs=2) as upool,
            tc.tile_pool(name="small", bufs=4) as spool,
            tc.tile_pool(name="psmain", bufs=5, space="PSUM") as ppool,
            tc.tile_pool(name="psxa", bufs=2, space="PSUM") as papool,
            tc.tile_pool(name="psptr", bufs=1, space="PSUM") as ptpool,
            tc.tile_pool(name="dram", bufs=1, space="DRAM") as dpool,
        ):
            # ================= constants =================
            ident = cpool.tile([128, 128], BF16, tag="ident")
            nc.sync.dma_start(ident[:], ident_in[:])

            # bias as bf16 (epilogue adds in f32; bias rounding ~2^-9*bias)
            bias_row = wfpool.tile([1, DOUT_C], F32, tag="wf", name="bias_row")
            nc.sync.dma_start(bias_row[:], bias_in[:])
            bias_rb = wfpool.tile([1, DOUT_C], BF16, tag="wf", name="bias_rb")
            nc.vector.tensor_copy(bias_rb[:], bias_row[:])
            bias_b = cpool.tile([128, DOUT_C], BF16, tag="bias_b")
            nc.gpsimd.partition_broadcast(bias_b[:], bias_rb[:])

            # ============ global |w| mean partials (wsc shard) ============
            wsums = cpool.tile([128, WSCT], F32, tag="wsums")
            wsc_tiles = []
            for t in range(WSCT):
                wst = wfpool.tile([128, DIN], F32, tag="wf", name=f"wsc{t}")
                eng = nc.sync if t % 2 == 0 else nc.scalar
                eng.dma_start(wst[:], wsc_in[t * 128:(t + 1) * 128, :])
                wsc_tiles.append(wst)
            for t in range(WSCT):
                nc.vector.tensor_reduce(
                    wsums[:, t:t + 1], wsc_tiles[t][:], axis=mybir.AxisListType.X,
                    op=mybir.AluOpType.add, apply_absolute_value=True,
                )
            wsum_p = cpool.tile([128, 1], F32, tag="wsum_p")
            nc.vector.tensor_reduce(
                wsum_p[:], wsums[:], axis=mybir.AxisListType.X,
                op=mybir.AluOpType.add,
            )
            wsum_b = cpool.tile([128, 1], F32, tag="wsum_b")
            nc.gpsimd.partition_all_reduce(
                wsum_b[:], wsum_p[:], channels=128,
                reduce_op=bass_isa.ReduceOp.add,
            )
            cc_in = dpool.tile([1, 1], F32)
            cc_out = dpool.tile([1, 1], F32)
            nc.sync.dma_start(cc_in[:], wsum_b[0:1, :])

            # ============ lora prep (no dependence on w_scale) ============
            la_s = wfpool.tile([128, KT, RANK], F32, tag="wf", name="la_s")
            nc.sync.dma_start(
                la_s[:], la_in.rearrange("(kt p) r -> p kt r", p=128)
            )
            amax0 = cpool.tile([128, 1], F32, tag="amax0")
            nc.vector.tensor_reduce(
                amax0[:], la_s[:], axis=mybir.AxisListType.XY,
                op=mybir.AluOpType.max, apply_absolute_value=True,
            )
            amax = cpool.tile([128, 1], F32, tag="amax")
            nc.gpsimd.partition_all_reduce(
                amax[:], amax0[:], channels=128, reduce_op=bass_isa.ReduceOp.max,
            )
            amax_c = cpool.tile([128, 1], F32, tag="amax_c")
            nc.vector.tensor_scalar(
                amax_c[:], amax[:], EPS, None, op0=mybir.AluOpType.max,
            )
            ia = cpool.tile([128, 1], F32, tag="ia")
            nc.vector.reciprocal(ia[:], amax_c[:])
            ia127 = cpool.tile([128, 1], F32, tag="ia127")
            nc.vector.tensor_scalar(
                ia127[:], ia[:], QMAX, None, op0=mybir.AluOpType.mult,
            )
            a_sc = cpool.tile([128, 1], F32, tag="a_sc")  # a_s = amax/127
            nc.vector.tensor_scalar(
                a_sc[:], amax_c[:], 1.0 / QMAX, None, op0=mybir.AluOpType.mult,
            )
            nc.vector.tensor_scalar(
                la_s[:], la_s[:], ia127[:], MAGIC,
                op0=mybir.AluOpType.mult, op1=mybir.AluOpType.add,
            )
            a_q = cpool.tile([128, KT, RANK], BF16, tag="a_q")
            nc.vector.tensor_scalar(
                a_q[:], la_s[:], -MAGIC, None, op0=mybir.AluOpType.add,
            )

            # lora_B absmax over the FULL tensor
            lbf_s = wfpool.tile([16, DOUT_FULL], F32, tag="wf", name="lbf_s")
            nc.sync.dma_start(lbf_s[:], lbf_in[:])
            bmax0 = cpool.tile([16, 1], F32, tag="bmax0")
            nc.vector.tensor_reduce(
                bmax0[:], lbf_s[:], axis=mybir.AxisListType.X,
                op=mybir.AluOpType.max, apply_absolute_value=True,
            )
            bmax = cpool.tile([16, 1], F32, tag="bmax")
            nc.gpsimd.partition_all_reduce(
                bmax[:], bmax0[:], channels=16, reduce_op=bass_isa.ReduceOp.max,
            )
            bmax_c = cpool.tile([16, 1], F32, tag="bmax_c")
            nc.vector.tensor_scalar(
                bmax_c[:], bmax[:], EPS, None, op0=mybir.AluOpType.max,
            )
            bmax_b = cpool.tile([128, 1], F32, tag="bmax_b")
            nc.gpsimd.partition_broadcast(bmax_b[:], bmax_c[0:1, :])
            ib = cpool.tile([128, 1], F32, tag="ib")
            nc.vector.reciprocal(ib[:], bmax_b[:])
            ib127 = cpool.tile([128, 1], F32, tag="ib127")
            nc.vector.tensor_scalar(
                ib127[:], ib[:], QMAX, None, op0=mybir.AluOpType.mult,
            )
            b_sc = cpool.tile([128, 1], F32, tag="b_sc")
            nc.vector.tensor_scalar(
                b_sc[:], bmax_b[:], 1.0 / QMAX, None, op0=mybir.AluOpType.mult,
            )
            # b_q2 = [b_q ; b_q]  (stacked for the K=32 lora stage-2 matmul)
            lb2_s = wfpool.tile([32, DOUT_C], F32, tag="wf", name="lb2_s")
            nc.sync.dma_start(lb2_s[0:16, :], lb_in[:])
            nc.sync.dma_start(lb2_s[16:32, :], lb_in[:])
            nc.vector.tensor_scalar(
                lb2_s[:], lb2_s[:], ib127[0:32, :], MAGIC,
                op0=mybir.AluOpType.mult, op1=mybir.AluOpType.add,
            )
            b_q2 = cpool.tile([32, DOUT_C], BF16, tag="b_q2")
            nc.vector.tensor_scalar(
                b_q2[:], lb2_s[:], -MAGIC, None, op0=mybir.AluOpType.add,
            )

            # ============ x pipeline ============
            xqT_l = [None] * MT
            xs_t_l = [None] * MT
            xsws_l = [None] * MT

            def emit_xsws(m):
                xsws = spool.tile([128, 1], F32, tag="xsws", bufs=8,
                                  name=f"xsws{m}")
                nc.vector.tensor_tensor(
                    xsws[:], xs_t_l[m][:], ws_t[:], op=mybir.AluOpType.mult,
                )
                xsws_l[m] = xsws

            def emit_x(m, defer_xsws=False):
                sxh = spool.tile([128, 2], F32, tag="sxh", name=f"sxh{m}")
                xqT = xtpool.tile([128, KT, 128], BF16, tag="xqT",
                                  name=f"xqT{m}")
                sx = spool.tile([128, 1], F32, tag="sx", name=f"sx{m}")
                sxc = spool.tile([128, 1], F32, tag="sxc", name=f"sxc{m}")
                xs_t = spool.tile([128, 1], F32, tag="xs_t", name=f"xs_t{m}")
                ix = spool.tile([128, 1], F32, tag="ix", name=f"ix{m}")
                xhs = []
                for h in range(2):
                    xh = xfpool.tile([128, DH], F32, tag="xf",
                                     name=f"x{m}h{h}")
                    nc.sync.dma_start(
                        xh[:], x_in[m * 128:(m + 1) * 128, h * DH:(h + 1) * DH])
                    nc.vector.tensor_reduce(
                        sxh[:, h:h + 1], xh[:], axis=mybir.AxisListType.X,
                        op=mybir.AluOpType.max, apply_absolute_value=True,
                    )
                    xhs.append(xh)
                nc.vector.tensor_reduce(
                    sx[:], sxh[:], axis=mybir.AxisListType.X,
                    op=mybir.AluOpType.max,
                )
                nc.vector.tensor_scalar(
                    sxc[:], sx[:], EPS, None, op0=mybir.AluOpType.max,
                )
                nc.vector.tensor_scalar(
                    xs_t[:], sxc[:], 1.0 / QMAX, None, op0=mybir.AluOpType.mult,
                )
                nc.vector.reciprocal(ix[:], xs_t[:])
                xs_t_l[m] = xs_t
                if not defer_xsws:
                    emit_xsws(m)
                for h in range(2):
                    xh = xhs[h]
                    nc.vector.tensor_scalar(
                        xh[:], xh[:], ix[:], MAGIC,
                        op0=mybir.AluOpType.mult, op1=mybir.AluOpType.add,
                    )
                    xqh = xqpool.tile([128, DH], BF16, tag="xq",
                                      name=f"xq{m}h{h}")
                    nc.vector.tensor_scalar(
                        xqh[:], xh[:], -MAGIC, None, op0=mybir.AluOpType.add,
                    )
                    nc.scalar.dma_start(
                        xqT[:, h * KTH:(h + 1) * KTH, :], xqh[:],
                        transpose=True,
                    )
                xqT_l[m] = xqT

            # ============ lora stage-1 + hi/lo pieces ============
            pcT_l = [None] * MT

            def emit_lora1(m):
                pxa = papool.tile([128, RANK], F32, tag="pxa", name=f"pxa{m}")
                for k in range(KT):
                    nc.tensor.matmul(
                        pxa[:], xqT_l[m][:, k, :], a_q[:, k, :],
                        start=(k == 0), stop=(k == KT - 1),
                    )
                v_xa = spool.tile([128, RANK], F32, tag="v_xa", name=f"vxa{m}")
                nc.vector.tensor_scalar(
                    v_xa[:], pxa[:], c2[:], None, op0=mybir.AluOpType.mult,
                )
                pieces = spool.tile([128, 2 * RANK], BF16, tag="pieces",
                                    name=f"pieces{m}")
                nc.vector.tensor_copy(pieces[:, 0:RANK], v_xa[:])
                hi_f = spool.tile([128, RANK], F32, tag="hi_f", name=f"hif{m}")
                nc.vector.tensor_copy(hi_f[:], pieces[:, 0:RANK])
                nc.vector.tensor_tensor(
                    pieces[:, RANK:2 * RANK], v_xa[:], hi_f[:],
                    op=mybir.AluOpType.subtract,
                )
                ptp = ptpool.tile([2 * RANK, 128], BF16, tag="ptp",
                                  name=f"ptp{m}")
                nc.tensor.transpose(ptp[:], pieces[:], ident[:])
                pcT = spool.tile([2 * RANK, 128], BF16, tag="pcT", bufs=6,
                                 name=f"pcT{m}")
                nc.scalar.copy(pcT[:], ptp[:])
                pcT_l[m] = pcT

            # first token tiles start loading/quantizing immediately
            # (quantization does not need w_scale; xsws products deferred)
            for m in range(6):
                emit_x(m, defer_xsws=True)

            # ============ AllReduce result pickup ============
            nc.gpsimd.collective_compute(
                "AllReduce", mybir.AluOpType.add,
                replica_groups=[list(range(N_CORES))],
                ins=[cc_in.opt()], outs=[cc_out.opt()],
            )
            wsg = cpool.tile([1, 1], F32, tag="wsg")
            # scalar queue is idle in the AR window; vector must stay free
            nc.scalar.dma_start(wsg[:], cc_out[:])
            wsg_b = cpool.tile([128, 1], F32, tag="wsg_b")
            nc.gpsimd.partition_broadcast(wsg_b[:], wsg[:])
            ws_t = cpool.tile([128, 1], F32, tag="ws_t")
            nc.vector.tensor_scalar(
                ws_t[:], wsg_b[:], 1.0 / float(N_FULL_W), EPS,
                op0=mybir.AluOpType.mult, op1=mybir.AluOpType.max,
            )
            inv_ws = cpool.tile([128, 1], F32, tag="inv_ws")
            nc.vector.reciprocal(inv_ws[:], ws_t[:])

            # c2 = a_s * b_s * SCALING / w_scale
            c2a = cpool.tile([128, 1], F32, tag="c2a")
            nc.vector.tensor_tensor(
                c2a[:], a_sc[:], b_sc[:], op=mybir.AluOpType.mult,
            )
            c2b = cpool.tile([128, 1], F32, tag="c2b")
            nc.vector.tensor_scalar(
                c2b[:], c2a[:], SCALING, None, op0=mybir.AluOpType.mult,
            )
            c2 = cpool.tile([128, 1], F32, tag="c2")
            nc.vector.tensor_tensor(
                c2[:], c2b[:], inv_ws[:], op=mybir.AluOpType.mult,
            )
            for m in range(6):
                emit_xsws(m)

            # ============ weight quant pipeline ============
            # wqT[c][p, sl, k, col] = w_q[dout=512c+128sl+col, din=128k+p]
            wqT = [
                wqpool.tile([128, 4, KT, 128], FP8, tag=f"wqT{c}",
                            name=f"wqT{c}")
                for c in range(NCH)
            ]

            def emit_w_chain(j):
                wt = wfpool.tile([128, DIN], F32, tag="wf", name=f"w{j}")
                nc.gpsimd.dma_start(wt[:], w_in[j * 128:(j + 1) * 128, :])
                # p1 (scalar, in-place): t = RNE_int(w*inv_ws) + MAGIC
                nc.scalar.activation(
                    wt[:], wt[:], mybir.ActivationFunctionType.Copy,
                    bias=MAGIC, scale=inv_ws[:],
                )
                # p2 (vector): shift to bf16 int window at OFF, clip high
                wqb = wbpool.tile([128, DIN], BF16, tag="wqb", name=f"wqb{j}")
                nc.vector.tensor_scalar(
                    wqb[:], wt[:], OFF - MAGIC, OFF + 1.0,
                    op0=mybir.AluOpType.add, op1=mybir.AluOpType.min,
                )
                wqTs = wspool.tile([128, KT, 128], BF16, tag="wqTs",
                                   name=f"wqTs{j}")
                nc.sync.dma_start(wqTs[:], wqb[:], transpose=True)
                # p3 (vector): clip low, remove offset, cast to fp8
                c, sl = j // 4, j % 4
                nc.vector.tensor_scalar(
                    wqT[c][:, sl, :, :], wqTs[:], OFF - 1.0, -OFF,
                    op0=mybir.AluOpType.max, op1=mybir.AluOpType.add,
                )

            # chunks 0,1 prepped before the waves; 2,3 interleaved into wave 0
            for j in range(8):
                emit_w_chain(j)

            emit_lora1(0)

            # ============ epilogue helper ============
            def emit_unit(m, c):
                ps = ppool.tile([128, 512], F32, tag="ps", name=f"ps{m}_{c}")
                for k in range(KT):
                    nc.tensor.matmul(
                        ps[:], xqT_l[m][:, k, :], wqT[c][:, :, k, :],
                        start=(k == 0), stop=False,
                    )
                nc.tensor.matmul(
                    ps[:], pcT_l[m][:], b_q2[:, c * 512:(c + 1) * 512],
                    start=False, stop=True,
                )
                u = upool.tile([128, 512], F32, tag="u", name=f"u{m}_{c}")
                nc.scalar.activation(
                    u[:], ps[:], mybir.ActivationFunctionType.Copy,
                    bias=0.0, scale=xsws_l[m][:],
                )
                nc.vector.tensor_tensor(
                    u[:], u[:], bias_b[:, c * 512:(c + 1) * 512],
                    op=mybir.AluOpType.add,
                )
                nc.scalar.dma_start(
                    out_d[m * 128:(m + 1) * 128, c * 512:(c + 1) * 512],
                    u[:],
                )

            # ============ chunk waves over the first NWAVE token tiles ====
            # consume chunks in arrival order so matmuls start as soon as
            # wqT[0] exists and never wait for the full weight prep.
            for c in range(NCH):
                for m in range(NWAVE):
                    emit_unit(m, c)
                    if c == 0:
                        if m < 2:
                            for j in range(8 + 4 * m, 12 + 4 * m):
                                emit_w_chain(j)
                        if m < 3:
                            emit_lora1(m + 1)
                    if c == 1 and m == 0:
                        emit_lora1(4)
                    if c == 2 and m == 0:
                        emit_lora1(5)
                    if c == 3:
                        # xqT ring slots free as wave-c3 units retire
                        if m + 6 < MT:
                            emit_x(m + 6)

            # ============ remaining token tiles, chunk-major ============
            for m in range(NWAVE, MT):
                for c in range(NCH):
                    emit_unit(m, c)
                    if c == 1 and m + 2 < MT:
                        emit_lora1(m + 2)
                if m + 6 < MT:
                    emit_x(m + 6)

    nc.compile()
    return nc


# ----------------------------------------------------------------------
# host-side wrapper
# ----------------------------------------------------------------------

@functools.lru_cache(maxsize=2)
def _get_nc(TOK, DIN, DOUT_C, WSC_ROWS, N_FULL_W):
    return build_nc(TOK, DIN, DOUT_C, WSC_ROWS, N_FULL_W)


def _prep(x, weight, bias, lora_A, lora_B):
    B, S, DIN = x.shape
    DOUT = weight.shape[0]
    NTOK = B * S
    assert NTOK % R_TOK == 0 and DOUT % C_OUT == 0 and DOUT % N_CORES == 0
    TOK = NTOK // R_TOK
    DOUT_C = DOUT // C_OUT
    WSC_ROWS = DOUT // N_CORES
    N_FULL_W = DOUT * DIN

    nc = _get_nc(TOK, DIN, DOUT_C, WSC_ROWS, N_FULL_W)

    x2 = np.ascontiguousarray(x.reshape(NTOK, DIN).astype(np.float32, copy=False))
    weight = np.ascontiguousarray(weight.astype(np.float32, copy=False))
    ident = np.eye(128, dtype=ml_dtypes.bfloat16)

    in_maps = []
    for core in range(N_CORES):
        i, j = core // C_OUT, core % C_OUT
        in_maps.append({
            "x_in": np.ascontiguousarray(x2[i * TOK:(i + 1) * TOK]),
            "w_in": np.ascontiguousarray(weight[j * DOUT_C:(j + 1) * DOUT_C]),
            "wsc_in": np.ascontiguousarray(
                weight[core * WSC_ROWS:(core + 1) * WSC_ROWS]),
            "bias_in": np.ascontiguousarray(
                bias[j * DOUT_C:(j + 1) * DOUT_C].reshape(1, DOUT_C)),
            "la_in": np.ascontiguousarray(lora_A.astype(np.float32, copy=False)),
            "lbf_in": np.ascontiguousarray(lora_B.astype(np.float32, copy=False)),
            "lb_in": np.ascontiguousarray(lora_B[:, j * DOUT_C:(j + 1) * DOUT_C]),
            "ident_in": ident,
        })
    return nc, in_maps, (B, S, NTOK, TOK, DOUT, DOUT_C)


def kernel(x, weight, bias, lora_A, lora_B):
    from concourse.bass_utils import run_bass_kernel_spmd

    nc, in_maps, (B, S, NTOK, TOK, DOUT, DOUT_C) = _prep(
        x, weight, bias, lora_A, lora_B)
    res = run_bass_kernel_spmd(nc, in_maps, core_ids=list(range(N_CORES)))

    out = np.empty((NTOK, DOUT), np.float32)
    for core in range(N_CORES):
        i, j = core // C_OUT, core % C_OUT
        out[i * TOK:(i + 1) * TOK, j * DOUT_C:(j + 1) * DOUT_C] = \
            res.results[core]["out"]
    return out.reshape(B, S, DOUT)


def _install_profile_shim():
    """Register the axon NTFF profile hook (antenv.axon_hooks is absent in
    this image; libaxon_pjrt.so supports the profile C ABI directly) and
    stub out the network-dependent artifact upload."""
    import types
    import ctypes
    import contextlib

    try:
        import antenv.axon_hooks  # noqa: F401
        have = True
    except ImportError:
        have = False
    if not have:
        so = "/opt/axon/libaxon_pjrt.so"
        lib = ctypes.CDLL(so)
        lib.axon_start_nrt_profile.argtypes = [
            ctypes.POINTER(ctypes.c_int64), ctypes.c_size_t]
        lib.axon_start_nrt_profile.restype = ctypes.c_int64
        lib.axon_stop_nrt_profile.argtypes = [ctypes.c_char_p]
        lib.axon_stop_nrt_profile.restype = ctypes.c_int64

        @contextlib.contextmanager
        def _hook(output_dir, device_ids):
            import jax
            jax.devices()
            if device_ids:
                ids = (ctypes.c_int64 * len(device_ids))(*device_ids)
                rc = lib.axon_start_nrt_profile(ids, len(device_ids))
            else:
                rc = lib.axon_start_nrt_profile(None, 0)
            if rc != 0:
                raise RuntimeError(f"axon_start_nrt_profile rc={rc}")
            try:
                yield
            finally:
                lib.axon_stop_nrt_profile(str(output_dir).encode())

        import antenv
        mod = types.ModuleType("antenv.axon_hooks")
        mod.get_axon_ntff_profile_hook = lambda: _hook
        mod.set_axon_ntff_profile_hook = lambda h: None
        sys.modules["antenv.axon_hooks"] = mod
        antenv.axon_hooks = mod

    from concourse import bass_utils
    bass_utils.upload_artifacts = lambda tmpdir: f"local:{tmpdir}"


def timed_run(inputs, trace_cores=None):
    """Run with NTFF tracing; returns max exec_time_ns across traced cores."""
    import tempfile
    _install_profile_shim()
    from concourse.bass_utils import run_bass_kernel_spmd

    nc, in_maps, _ = _prep(**inputs)
    res = run_bass_kernel_spmd(
        nc, in_maps, core_ids=list(range(N_CORES)), trace=True,
        trace_cores=trace_cores if trace_cores is not None
        else list(range(N_CORES)),
        tmpdir=tempfile.mkdtemp(prefix="dyadic_trace_"),
    )
    return res.exec_time_ns
